# revision 42
# baseline (speedup 1.0000x reference)
"""Multi-head causal self-attention on 8 TRN2 NeuronCores.

Problem (hardcoded): x[2,2048,1024] f32, Q/K/V/O [1024,1024] f32, 16 heads,
Dh=64, causal softmax, out = attn(x) @ O.T  -> [2,2048,1024] f32.

Sharding: core c handles batch b=c//4 and head group g=c%4 (4 heads each).
Each core computes a partial output (its heads' contribution through the O
projection); the host gather sums the 4 partials per batch (the all-reduce
of the hint, performed at unshard time).

Device algorithm per core (heads h=0..3):
  Projections (fp32r) into per-head augmented tiles:
      qaug[h][0:64, s] = (Q_h/8) @ x.T ; row 64 <- -rowmax (DMA'd later)
      kaug[h][0:64, s] = K_h @ x.T     ; row 64 = ones
      v[s,d] bf16 (+ ones column for free softmax denominators)
  A-pass per head (i = q-block): scores[q,k] fp32r chunks -> PSUM; diag
      causal mask added via PE (-BIG upper); DVE reduce_max per fill (+
      small combine) -> negmax[q,i]; negmax [128,16] -PE transpose->
      [16,128] -> DMA to qaug row 64.
  T-pass per head (j = k-block): sT[k,q] = ONE fp32r matmul with K=65:
      rows 0..63 give q.k, row 64 gives 1*(-m_q) ==> s - m in one pass.
      ACT exp -> PT bf16; gpsimd affine_select zeroes the upper triangle of
      the (j,j) block (invalid entries may be inf pre-mask; replaced, never
      summed).
  Schedule (engines balanced per window; DVE max-chain paced globally):
      proj0 | RR{A0+A1 fills, proj1 subwaves, vproj} |
      T0+av0+Afills | T1+av1+Afills | T2+av2+Afills+hoT01 |
      T3+av3+hoT23+oproj+outdma (lagged)
  av(h,j): out[q,65] = PT.T @ [v|1]; DVE reciprocal+scalar-mul -> ho[j].
  Output: ho -PE transpose-> hoT (+ACT/DVE copy); out = hoT.T @ O_cols.T;
      PSUM->SBUF copy -> DMA out, all lagged so no PE wait-queue clogs.
"""
import numpy as np

import concourse.bass as bass
import concourse.tile as tile
from concourse import bacc, mybir
from concourse.bass_utils import run_bass_kernel_spmd
from concourse.masks import make_identity

F32 = mybir.dt.float32
F32R = mybir.dt.float32r
FP16 = mybir.dt.float16
BF16 = mybir.dt.bfloat16

B, S, D, H = 2, 2048, 1024, 16
DH = 64          # head dim
HPC = 4          # heads per core
NB = S // 128    # 16 q/k blocks
FT = D // 128    # 8 f-tiles
NEG = -3.0e38

# PT column offsets: head-local P^T storage, block j spans q-cols [j*128, S)
PT_OFF = [0] * (NB + 1)
for _j in range(NB):
    PT_OFF[_j + 1] = PT_OFF[_j] + (S - _j * 128)
PT_COLS = PT_OFF[NB]  # 17408


def _splits(w, cap):
    """Split w (multiple of 128) into balanced 128-granular pieces <= cap."""
    n = -(-w // cap)
    q, r = divmod(w // 128, n)
    return [128 * (q + (1 if k < r else 0)) for k in range(n)]


def build_nc():
    nc = bacc.Bacc(None, target_bir_lowering=False, debug=False)

    xt_d = nc.dram_tensor("xt", [D, S], FP16, kind="ExternalInput")
    qt_d = nc.dram_tensor("qt", [D, 256], FP16, kind="ExternalInput")
    kt_d = nc.dram_tensor("kt", [D, 256], FP16, kind="ExternalInput")
    vt_d = nc.dram_tensor("vt", [D, 256], FP16, kind="ExternalInput")
    ot_d = nc.dram_tensor("ot", [256, D], FP16, kind="ExternalInput")
    tri_d = nc.dram_tensor("tri", [128, 128], F32, kind="ExternalInput")
    rsh_d = nc.dram_tensor("rsh", [128, 128], F32, kind="ExternalInput")
    out_d = nc.dram_tensor("out", [S, D], BF16, kind="ExternalOutput")

    with tile.TileContext(nc) as tc:
        with (
            tc.tile_pool(name="mid", bufs=1) as mid,
            tc.tile_pool(name="ph2", bufs=2) as ph2,
        ):
            # whole-kernel tensors
            qaug = [mid.tile([65, S], F32R, name=f"qaug{h}") for h in range(HPC)]
            kaug = [mid.tile([65, S], F32R, name=f"kaug{h}") for h in range(HPC)]
            v_sb = [mid.tile([128, HPC, 65], FP16, name=f"v{j}")
                    for j in range(NB)]
            ho_sb = [mid.tile([128, 256], FP16, name=f"ho{j}") for j in range(NB)]
            hot_sb = [mid.tile([128, 256], FP16, name=f"hot{j}") for j in range(NB)]
            ot_sb = mid.tile([128, 2, D], FP16)
            tri_sb = mid.tile([128, 128], BF16)
            rsh_sb = mid.tile([128, 128], BF16)
            identf = mid.tile([128, 128], F32)
            onesrow = mid.tile([16, 128], F32)
            onesrow_r = mid.tile([16, 128], F32R)
            identb = mid.tile([128, 128], FP16)
            negmaxs = [mid.tile([128, NB], F32, name=f"negmax{h}")
                       for h in range(HPC)]

            # ---------------- Phase 1: projections ----------------
            ph1_cm = tc.tile_pool(name="ph1", bufs=1)
            psAe_cm = tc.tile_pool(name="psAe", bufs=2, space="PSUM")
            pp_cm = tc.tile_pool(name="pp", bufs=4, space="PSUM")
            ph1 = ph1_cm.__enter__()
            psAe = psAe_cm.__enter__()
            pp = pp_cm.__enter__()

            xt_sb = [ph1.tile([128, S], FP16, name=f"xt_sb{t}")
                     for t in range(FT)]
            qtw = ph1.tile([128, FT, 256], FP16)
            ktw = ph1.tile([128, FT, 256], FP16)
            vtw = ph1.tile([128, FT, 256], FP16)
            # DMA triggers first (in-order queues): inputs before memsets.
            nc.sync.dma_start(qtw[:], qt_d[:].rearrange("(t p) m -> p t m", p=128))
            nc.sync.dma_start(ktw[:], kt_d[:].rearrange("(t p) m -> p t m", p=128))
            for t in range(FT):
                nc.sync.dma_start(xt_sb[t][:], xt_d[t * 128:(t + 1) * 128, :])
            nc.sync.dma_start(vtw[:], vt_d[:].rearrange("(t p) m -> p t m", p=128))
            nc.gpsimd.dma_start(tri_sb[:], tri_d[:])
            nc.gpsimd.dma_start(rsh_sb[:], rsh_d[:])
            nc.sync.dma_start(ot_sb[:], ot_d[:].rearrange("(t p) n -> p t n", p=128))

            make_identity(nc, identf[:])
            make_identity(nc, identb[:])
            nc.gpsimd.memset(onesrow[:], 1.0)
            nc.vector.tensor_copy(onesrow_r[:], onesrow[:])
            for h in range(HPC):
                nc.sync.dma_start(kaug[h][64:65, :], onesrow_r[:])
            for j in range(NB):
                nc.vector.memset(v_sb[j][:, :, 64:65], 1.0)

            def emit_proj_subwave(p, chains2):
                """One t-major pass of 2 chains; chains2 = [(c, w_sb, dstl)].
                Pair p covers heads 2p (psum rows 0:64) and 2p+1 (64:128)."""
                work = []
                for c, w_sb, dstl in chains2:
                    ps = pp.tile([128, 512], F32, tag="ps", name="ps")
                    work.append((ps, w_sb, dstl, c))
                for t in range(FT):
                    for ps, w_sb, dstl, c in work:
                        nc.tensor.matmul(
                            ps[:],
                            w_sb[:, t, p * 128:(p + 1) * 128],
                            xt_sb[t][:, c * 512:(c + 1) * 512],
                            start=(t == 0), stop=(t == FT - 1),
                        )
                for ps, w_sb, dstl, c in work:
                    cols = slice(c * 512, (c + 1) * 512)
                    nc.scalar.copy(dstl[2 * p][0:64, cols], ps[0:64, :])
                    nc.scalar.copy(dstl[2 * p + 1][0:64, cols], ps[64:128, :])

            def proj_subwaves(p):
                out = []
                for c in range(4):
                    out.append(lambda p=p, c=c: emit_proj_subwave(
                        p, [(c, qtw, qaug), (c, ktw, kaug)]))
                return out

            def emit_vproj_block(sb_i):
                ps = pp.tile([128, 512], F32, tag="ps", name="vps")
                for t in range(FT):
                    nc.tensor.matmul(
                        ps[:, 0:256],
                        xt_sb[t][:, sb_i * 128:(sb_i + 1) * 128],
                        vtw[:, t, :],
                        start=(t == 0), stop=(t == FT - 1),
                    )
                nc.scalar.copy(
                    v_sb[sb_i][:, :, 0:64],
                    ps[:, 0:256].rearrange("p (h d) -> p h d", d=64),
                )

            # ---------------- A-pass: scores -> row max ----------------
            def emit_A_fill_list(h, pool, cap=512):
                out = []
                negmax = negmaxs[h]
                for i in range(NB):
                    w = (i + 1) * 128
                    fl = []
                    _r = w
                    while _r > 0:
                        fl.append(min(cap, _r))
                        _r -= fl[-1]
                    mph = {}  # lazily allocated at emission time (WAR order)
                    col0 = 0
                    for fi, fw in enumerate(fl):
                        def emit(h=h, i=i, fi=fi, fw=fw, fl=fl, mph=mph,
                                 col0=col0, pool=pool, cap=cap):
                            if len(fl) > 1 and fi == 0:
                                mph["mp"] = ph2.tile(
                                    [128, 4], F32, tag="mp",
                                    name=f"mp{h}_{i}", bufs=4)
                            mp = mph.get("mp")
                            ps = pool.tile([128, cap], F32, tag="sA",
                                           name=f"sA{h}")
                            c0 = 0
                            while c0 < fw:
                                cw = min(512, fw - c0)
                                nc.tensor.matmul(
                                    ps[:, c0:c0 + cw],
                                    qaug[h][0:64, i * 128:(i + 1) * 128],
                                    kaug[h][0:64, col0 + c0:col0 + c0 + cw],
                                    start=True, stop=True,
                                )
                                c0 += cw
                            if fi == len(fl) - 1:  # diag: += -BIG*[k>q]
                                nc.tensor.matmul(
                                    ps[:, fw - 128:fw],
                                    rsh_sb[:], tri_sb[:],
                                    start=False, stop=True,
                                    skip_group_check=True)
                            if len(fl) == 1:
                                nc.vector.reduce_max(
                                    negmax[:, i:i + 1], ps[:, 0:fw],
                                    axis=mybir.AxisListType.X, negate=True)
                            else:
                                nc.vector.reduce_max(
                                    mp[:, fi:fi + 1], ps[:, 0:fw],
                                    axis=mybir.AxisListType.X)
                                if fi == len(fl) - 1:
                                    nc.vector.reduce_max(
                                        negmax[:, i:i + 1], mp[:, 0:len(fl)],
                                        axis=mybir.AxisListType.X, negate=True)
                        out.append(emit)
                        col0 += fw
                return out

            def emit_negrow(h, pool):
                # negmax [128,16] -> [16,128] -> DMA into qaug[h] row 64
                pst = pool.tile([16, 128], F32, tag="sA", name="pst")
                nc.tensor.transpose(pst[:], negmaxs[h][:], identf[:])
                stage = ph2.tile([16, 128], F32R, tag="stage", bufs=4)
                nc.vector.tensor_copy(stage[:], pst[:])
                nc.sync.dma_start(qaug[h][64:65, :], stage[:])

            # ---------------- T-pass / av / out ----------------
            pts = {}

            def emit_T_block(h, j):
                pt = pts[h]
                base = PT_OFF[j]
                col0 = j * 128
                done = 0
                W = S - j * 128
                while done < W:
                    fw = min(1024, W - done)
                    ps = psT.tile([128, 1024], F32, tag="sT", name=f"sT{h}")
                    c0 = 0
                    while c0 < fw:
                        # matmul outputs must stay within a 512-col PSUM bank
                        cw = min(512, fw - c0)
                        nc.tensor.matmul(
                            ps[:, c0:c0 + cw],
                            kaug[h][0:65, j * 128:(j + 1) * 128],
                            qaug[h][0:65, col0 + done + c0:col0 + done + c0 + cw],
                            start=True, stop=True,
                        )
                        c0 += cw
                    nc.scalar.activation(
                        pt[:, base + done:base + done + fw],
                        ps[:, 0:fw],
                        mybir.ActivationFunctionType.Exp)
                    done += fw
                # zero upper triangle of the diag block (keep q >= k);
                # pre-mask entries may be inf — replaced, never summed
                nc.gpsimd.affine_select(
                    out=pt[:, base:base + 128],
                    in_=pt[:, base:base + 128],
                    compare_op=mybir.AluOpType.is_ge,
                    fill=0.0,
                    base=0,
                    pattern=[[1, 128]],
                    channel_multiplier=-1,
                )

            def emit_av(h, j):
                pt = pts[h]
                av = psV.tile([128, 65], F32, tag="av", name="av")
                for jp in range(j + 1):
                    nc.tensor.matmul(
                        av[:],
                        pt[:, PT_OFF[jp] + (j - jp) * 128:
                           PT_OFF[jp] + (j - jp) * 128 + 128],
                        v_sb[jp][:, h, :],
                        start=(jp == 0), stop=(jp == j),
                    )
                recip = ph2.tile([128, 1], F32, tag="recip", bufs=6)
                nc.vector.reciprocal(recip[:], av[:, 64:65])
                nc.vector.tensor_scalar_mul(
                    ho_sb[j][:, h * 64:(h + 1) * 64],
                    av[:, 0:64], recip[:])

            def emit_hoT(j, t, eng):
                # ho[j] cols t*128.. -> PE transpose -> PSUM -> copy to hot
                ptile = psAl.tile([128, 128], FP16, tag="sA", name="ptile")
                nc.tensor.transpose(
                    ptile[:], ho_sb[j][:, t * 128:(t + 1) * 128], identb[:])
                if eng == "act":
                    nc.scalar.copy(hot_sb[j][:, t * 128:(t + 1) * 128],
                                   ptile[:])
                else:
                    nc.vector.tensor_copy(hot_sb[j][:, t * 128:(t + 1) * 128],
                                          ptile[:])

            ostages = {}

            def emit_oproj(j):
                hot = hot_sb[j]
                ostage = ph2.tile([128, D], BF16, tag="ostage", name="ostage",
                                  bufs=4)
                ostages[j] = ostage
                for nchunk in range(2):
                    pot = psAl.tile([128, 512], F32, tag="sA", name="pot")
                    for t in range(2):
                        nc.tensor.matmul(
                            pot[:],
                            hot[:, t * 128:(t + 1) * 128],
                            ot_sb[:, t, nchunk * 512:(nchunk + 1) * 512],
                            start=(t == 0), stop=(t == 1),
                        )
                    if nchunk == 0:
                        nc.scalar.copy(
                            ostage[:, nchunk * 512:(nchunk + 1) * 512], pot[:])
                    else:
                        nc.vector.tensor_copy(
                            ostage[:, nchunk * 512:(nchunk + 1) * 512], pot[:])

            def emit_outdma(j):
                # separate from emit_oproj: by dispatch time its waits are
                # resolved, so it never head-of-line blocks the SP queue.
                nc.sync.dma_start(out_d[j * 128:(j + 1) * 128, :], ostages[j])

            # ---------------- emission schedule ----------------
            # pre-T0: proj0 subwaves first (A0/A1 = heads of pair 0 depend
            # only on proj0), then round-robin proj1 subwaves + vproj blocks
            # with the A0/A1 score fills: the DVE max chain starts early
            # while PE stays dense through the DMA-bound front.
            for sw in proj_subwaves(0):
                sw()
            gq = emit_A_fill_list(0, psAe, cap=1024)
            gq.append(lambda: emit_negrow(0, psAe))
            gq += emit_A_fill_list(1, psAe, cap=1024)
            gq.append(lambda: emit_negrow(1, psAe))
            units = proj_subwaves(1) + \
                [(lambda b=b: emit_vproj_block(b)) for b in range(NB)]
            cursor = 0
            for u in units:
                u()
                for _ in range(6):
                    if cursor < len(gq):
                        gq[cursor]()
                        cursor += 1
            while cursor < len(gq):
                gq[cursor]()
                cursor += 1

            pp_cm.__exit__(None, None, None)
            psAe_cm.__exit__(None, None, None)
            ph1_cm.__exit__(None, None, None)   # frees xt/weights SBUF

            pt_cm = tc.tile_pool(name="pt_pool", bufs=2)
            psAl_cm = tc.tile_pool(name="psAl", bufs=2, space="PSUM")
            psT_cm = tc.tile_pool(name="psT", bufs=2, space="PSUM")
            psV_cm = tc.tile_pool(name="psV", bufs=2, space="PSUM")
            pt_pool = pt_cm.__enter__()
            psAl = psAl_cm.__enter__()
            psT = psT_cm.__enter__()
            psV = psV_cm.__enter__()

            # global A-fill queue for heads 2,3, drained 2 items per T block
            # across segments T0..T2 (negrows land with plenty of slack)
            gq2 = emit_A_fill_list(2, psAl)
            gq2.append(lambda: emit_negrow(2, psAl))
            gq2 += emit_A_fill_list(3, psAl)
            gq2.append(lambda: emit_negrow(3, psAl))
            gqc = [0]

            def drain(n):
                for _ in range(n):
                    if gqc[0] < len(gq2):
                        gq2[gqc[0]]()
                        gqc[0] += 1

            def seg(hT, hoT01=False):
                pts[hT] = pt_pool.tile([128, PT_COLS], FP16, tag="pt",
                                       name=f"pt{hT}")
                for j in range(NB):
                    emit_T_block(hT, j)
                    if j >= 1:
                        emit_av(hT, j - 1)
                    drain(2)
                    if hoT01 and j >= 2:
                        emit_hoT(j - 2, 0, "dve")
                emit_av(hT, NB - 1)
                if hoT01:
                    emit_hoT(NB - 2, 0, "dve")
                    emit_hoT(NB - 1, 0, "dve")

            seg(0)
            seg(1)
            seg(2, hoT01=True)
            drain(len(gq2))  # safety: nothing should remain

            # T3: av3 lag-1, hoT(t=1) lag-2, O-proj lag-4, out-DMA lag-6 —
            # long-latency deps must be resolved before their PE/SP
            # consumers dispatch, or they clog the shallow wait queues.
            pts[3] = pt_pool.tile([128, PT_COLS], FP16, tag="pt", name="pt3")
            for j in range(NB):
                emit_T_block(3, j)
                if j >= 1:
                    emit_av(3, j - 1)
                if j >= 2:
                    emit_hoT(j - 2, 1, "dve")
                if j >= 4:
                    emit_oproj(j - 4)
                if j >= 6:
                    emit_outdma(j - 6)
            emit_av(3, NB - 1)
            emit_hoT(NB - 2, 1, "dve")
            emit_oproj(NB - 4)
            emit_outdma(NB - 6)
            emit_oproj(NB - 3)
            emit_outdma(NB - 5)
            emit_hoT(NB - 1, 1, "dve")
            emit_oproj(NB - 2)
            emit_outdma(NB - 4)
            emit_oproj(NB - 1)
            emit_outdma(NB - 3)
            emit_outdma(NB - 2)
            emit_outdma(NB - 1)

            for cm in (psV_cm, psT_cm, psAl_cm, pt_cm):
                cm.__exit__(None, None, None)

    nc.compile()
    return nc


_NC_CACHE = None


def _get_nc():
    global _NC_CACHE
    if _NC_CACHE is None:
        _NC_CACHE = build_nc()
    return _NC_CACHE


def kernel(x, Q, K, V, O, num_heads=16, _want_results=False, **run_kwargs):
    x = np.asarray(x, dtype=np.float32)
    Q = np.asarray(Q, dtype=np.float32)
    K = np.asarray(K, dtype=np.float32)
    V = np.asarray(V, dtype=np.float32)
    O = np.asarray(O, dtype=np.float32)
    assert x.shape == (B, S, D) and int(num_heads) == H

    idx = np.arange(128)
    # tri[c,k] = [c<=k]; rsh[c,q] = -BIG*[c==q+1]
    # A-side: (rsh.T@tri)[q,k] = -BIG*[k>q]
    tri = (idx[:, None] <= idx[None, :]).astype(np.float32)
    rsh = np.zeros((128, 128), dtype=np.float32)
    rsh[idx[1:], idx[:-1]] = NEG

    in_maps = []
    for c in range(8):
        b, g = c // 4, c % 4
        rows = slice(g * 256, (g + 1) * 256)
        in_maps.append(dict(
            xt=np.ascontiguousarray(x[b].T).astype(np.float16),
            qt=np.ascontiguousarray((Q[rows, :] / 8.0).T).astype(np.float16),
            kt=np.ascontiguousarray(K[rows, :].T).astype(np.float16),
            vt=np.ascontiguousarray(V[rows, :].T).astype(np.float16),
            ot=np.ascontiguousarray(O[:, rows].T).astype(np.float16),
            tri=tri,
            rsh=rsh,
        ))

    nc = _get_nc()
    res = run_bass_kernel_spmd(nc, in_maps, core_ids=list(range(8)), **run_kwargs)

    out = np.zeros((B, S, D), dtype=np.float32)
    for c in range(8):
        out[c // 4] += np.asarray(res.results[c]["out"], dtype=np.float32)
    if _want_results:
        return out, res
    return out
